# revision 33
# baseline (speedup 1.0000x reference)
"""Trainium2 Bass kernel for the LocalGNOBlock (windowed GNN message passing).

Math restructuring (vs the naive 12x full MLP evaluations):
  msg first layer is linear over concat([h_i, h_j, dc]):
      z_d[i] = (A - C)[i] + (B + C)[i+d] + b1,  d in {+-1..+-6}
  where A = h @ W1a, B = h @ W1b, C = coord x w1c (rank-1).
  The second msg layer is summed over edges BEFORE the matmul:
      agg_pre = (sum_d silu(z_d)) @ W2
  Aggregate divide-by-count folds into W2 (interior count == 12) with a
  6-column fixup at each sequence end.  LayerNorm stats are computed with
  ones-vector matmuls (channel dim lives on partitions).

Pass 2 (normalize) is matmul-free: mu/rstd rows are round-tripped through
DRAM and broadcast to 128 partitions with stride-0 DMA; the normalize is
two fp16 2x-mode tensor_tensor ops plus one 4x-mode tensor_scalar with
per-partition (g, b) operands.  Output is written fp16 and cast on host.

Sharding: batch dim B=8 -> one batch element per NeuronCore (no halo needed).
"""

import numpy as np

K = 6
HID = 128
N = 16384
B = 8
EPS = 1e-5
T = 512                 # token chunk (matmul + elementwise granularity)
NCH = N // T            # 32 chunks
OFF0 = 8                # D_full column of token 0 (even, for fp16 alignment)
NCOL = N + 2 * OFF0     # D_full width

# offsets: even offsets read D_A, odd offsets read D_B (shifted copy)
NEG_EVEN = [-6, -4, -2]
ODD = [-5, -3, -1, 1, 3, 5]
POS_EVEN = [2, 4, 6]
SEG_ORDER = NEG_EVEN + ODD + POS_EVEN  # 12 segments in Z

_compiled = None


def _build_bass(dt_act):
    import concourse.bacc as bacc
    import concourse.bass as bass
    import concourse.tile as tile
    from concourse import mybir

    f32 = mybir.dt.float32
    DT = dt_act

    nc = bacc.Bacc("TRN2", target_bir_lowering=False, debug=False)

    # ---- DRAM I/O ----
    hT = nc.dram_tensor("hT", [HID, N], DT, kind="ExternalInput")
    coordR = nc.dram_tensor("coordR", [1, N], DT, kind="ExternalInput")
    # wall packs the eight [128,128] weights: W1b W1a W2U U1a U2 ident W2s U1b
    wall = nc.dram_tensor("wall", [HID, 8 * HID], DT, kind="ExternalInput")
    # rows packs [w1c | -w1c | b2u] ; cols packs [b1 | bu | g | b] (f32)
    rows = nc.dram_tensor("rows", [1, 3 * HID], DT, kind="ExternalInput")
    cols = nc.dram_tensor("cols", [HID, 4], f32, kind="ExternalInput")
    fixb = nc.dram_tensor("fixb", [1, 2 * K], f32, kind="ExternalInput")  # 12/count head|tail
    # band-select matrix: hot column = 1/128 (stats row packing)
    selb = nc.dram_tensor("selb", [HID, 2 * 2 * NCH - 1], DT, kind="ExternalInput")
    outT = nc.dram_tensor("outT", [HID, N], DT, kind="ExternalOutput")

    Silu = mybir.ActivationFunctionType.Silu
    Sqrt = mybir.ActivationFunctionType.Sqrt

    with tile.TileContext(nc) as tc:
        with (
            tc.tile_pool(name="singles", bufs=1) as singles,
            tc.tile_pool(name="big", bufs=1) as big,
            tc.tile_pool(name="work", bufs=1) as work,
            tc.tile_pool(name="zpool", bufs=3) as zpool,
            tc.tile_pool(name="opool", bufs=3) as opool,
            tc.tile_pool(name="dramp", bufs=1, space="DRAM") as dramp,
            tc.tile_pool(name="psA", bufs=1, space="PSUM") as psA,
            tc.tile_pool(name="psB", bufs=1, space="PSUM") as psB,
            tc.tile_pool(name="psS", bufs=1, space="PSUM") as psS,
        ):
            # ---- constants into SBUF (few large DMAs, spread across queues) ----
            swall = singles.tile([HID, 8 * HID], DT)
            nc.scalar.dma_start(out=swall, in_=wall[:, :])
            sW1b = swall[:, 0 * HID:1 * HID]
            sW1a = swall[:, 1 * HID:2 * HID]
            sW2U = swall[:, 2 * HID:3 * HID]
            sU1a = swall[:, 3 * HID:4 * HID]
            sU2 = swall[:, 4 * HID:5 * HID]
            sIdent = swall[:, 5 * HID:6 * HID]
            sW2s = swall[:, 6 * HID:7 * HID]
            sU1b = swall[:, 7 * HID:8 * HID]
            srows = singles.tile([1, 3 * HID], DT)
            nc.gpsimd.dma_start(out=srows, in_=rows[:, :])
            sw1c = srows[:, 0:HID]
            sw1cn = srows[:, HID:2 * HID]
            sb2u = srows[:, 2 * HID:3 * HID]
            scols = singles.tile([HID, 4], f32)
            nc.gpsimd.dma_start(out=scols, in_=cols[:, :])
            sb1 = scols[:, 0:1]
            sbu = scols[:, 1:2]
            sg = scols[:, 2:3]
            sb = scols[:, 3:4]
            ssel = singles.tile([HID, 2 * 2 * NCH - 1], DT)
            nc.scalar.dma_start(out=ssel, in_=selb[:, :])

            def bcast_rows(a, p=HID):
                return bass.AP(tensor=a.tensor, offset=a.offset,
                               ap=[[0, p]] + list(a.ap[1:]))

            # broadcast [1,12] -> [128,12] fix tile (head | tail)
            sfix = singles.tile([HID, 2 * K], f32)
            nc.gpsimd.dma_start(out=sfix, in_=bcast_rows(fixb[0:1, :]))
            sfixf = sfix[:, 0:K]
            sfixl = sfix[:, K:2 * K]
            sones_row = singles.tile([1, T], DT)
            nc.vector.memset(sones_row, 1.0)
            seps = singles.tile([NCH, 1], f32)
            nc.vector.memset(seps, float(EPS))

            # ---- big persistent buffers ----
            D_A = big.tile([HID, NCOL], DT)      # token j at col OFF0 + j
            D_B = big.tile([HID, NCOL], DT)      # token j at col OFF0 + 1 + j
            x_full = big.tile([HID, N], DT)
            # zero halo columns of D so boundary silu stays finite
            nc.vector.memset(D_A[:, 0:OFF0], 0.0)
            nc.vector.memset(D_A[:, OFF0 + N:NCOL], 0.0)
            nc.vector.memset(D_B[:, 0:OFF0 + 1], 0.0)
            nc.vector.memset(D_B[:, OFF0 + 1 + N:NCOL], 0.0)

            # LN stats: rows [0:32] = E[x]/chunk, [32:64] = E[x^2]/chunk
            st_ps = psS.tile([2 * NCH, T], f32)

            hts = {}
            crd = {}

            def load_chunk(c):
                ht = work.tile([HID, T], DT, tag="ht", bufs=6)
                nc.sync.dma_start(out=ht, in_=hT[:, c * T:(c + 1) * T])
                co = work.tile([1, T], DT, tag="co", bufs=6)
                nc.sync.dma_start(out=co, in_=coordR[:, c * T:(c + 1) * T])
                hts[c] = ht
                crd[c] = co

            def phase_a(c):
                # D chunk = W1b.T @ h  +  w1c x coord   (PSUM accumulate)
                d_ps = psA.tile([HID, T], f32, tag="de", bufs=2)
                nc.tensor.matmul(d_ps, sW1b, hts[c], start=True, stop=False)
                nc.tensor.matmul(d_ps, sw1c, crd[c], start=False, stop=True)
                col = OFF0 + c * T
                nc.vector.tensor_copy(D_A[:, col:col + T], d_ps)
                # shifted copy via SBUF->SBUF DMA: gpsimd COPY would contend
                # with the DVE for the shared POOL SBUF port and halve DVE
                # throughput on overlapping ops
                nc.gpsimd.dma_start(
                    out=D_B[:, col + 1:col + 1 + T], in_=D_A[:, col:col + T])

            def seg_in1(tile_ap, col, nseg):
                # [128, nseg, T] AP over D with outer column-stride 2
                s = tile_ap[:, col:col + T]
                return bass.AP(tensor=s.tensor, offset=s.offset,
                               ap=[s.ap[0], [2, nseg], [1, T]])

            def phase_b(t):
                ht, co = hts[t], crd[t]
                # E chunk = W1a.T @ h - w1c x coord
                e_ps = psA.tile([HID, T], f32, tag="de", bufs=2)
                nc.tensor.matmul(e_ps, sW1a, ht, start=True, stop=False)
                nc.tensor.matmul(e_ps, sw1cn, co, start=False, stop=True)
                e_sb = work.tile([HID, T], DT, tag="esb", bufs=3)
                nc.vector.tensor_copy(e_sb, e_ps)

                # Z: 12 segments of E + shifted D, 3 stride-2 groups
                z = zpool.tile([HID, 12 * T], DT, tag="z")
                zv = z.rearrange("p (s t) -> p s t", t=T)

                def e_bc(nseg):
                    return bass.AP(tensor=e_sb.tensor, offset=e_sb.offset,
                                   ap=[e_sb.ap[0], [0, nseg], [1, T]])
                base = t * T
                groups = [
                    (D_A, OFF0 + base + NEG_EVEN[0], 0, 3),
                    (D_B, OFF0 + 1 + base + ODD[0], 3, 6),
                    (D_A, OFF0 + base + POS_EVEN[0], 9, 3),
                ]
                for dbuf, col, s0, nseg in groups:
                    nc.vector.tensor_tensor(
                        out=zv[:, s0:s0 + nseg, :],
                        in0=e_bc(nseg), in1=seg_in1(dbuf, col, nseg),
                        op=mybir.AluOpType.add)

                # silu over all 12 segments at once (bias = msg_b1)
                nc.scalar.activation(z, z, Silu, bias=sb1, scale=1.0)

                # zero invalid boundary columns (torn edges of the sequence)
                if t == 0:
                    for s, d in enumerate(SEG_ORDER):
                        if d < 0:
                            nc.vector.memset(zv[:, s, 0:-d], 0.0)
                if t == NCH - 1:
                    for s, d in enumerate(SEG_ORDER):
                        if d > 0:
                            nc.vector.memset(zv[:, s, T - d:T], 0.0)

                # update-MLP first layer.  For interior chunks the agg path is
                # algebraically folded: U1b.T @ (W2s.T @ sum_s silu(z_s)) ==
                # (W2s @ U1b).T @ sum_s silu(z_s), so the 12 segment matmuls
                # accumulate straight into u_ps with lhsT = W2U.  Boundary
                # chunks need the 12/count fixup on agg, so they keep the
                # explicit agg tensor.
                u_ps = psA.tile([HID, T], f32, tag="upd", bufs=2)
                if 0 < t < NCH - 1:
                    nc.tensor.matmul(u_ps, sU1a, ht, start=True, stop=False)
                    for s in range(12):
                        nc.tensor.matmul(u_ps, sW2U, zv[:, s, :],
                                         start=False, stop=(s == 11))
                else:
                    a_ps = psB.tile([HID, T], f32, tag="agg", bufs=1)
                    for s in range(12):
                        nc.tensor.matmul(a_ps, sW2s, zv[:, s, :],
                                         start=(s == 0), stop=(s == 11))
                    agg = work.tile([HID, T], DT, tag="agg_sb", bufs=2)
                    nc.vector.tensor_copy(agg, a_ps)
                    if t == 0:
                        nc.vector.tensor_tensor(out=agg[:, 0:K], in0=a_ps[:, 0:K],
                                                in1=sfixf, op=mybir.AluOpType.mult)
                    if t == NCH - 1:
                        nc.vector.tensor_tensor(out=agg[:, T - K:T],
                                                in0=a_ps[:, T - K:T],
                                                in1=sfixl, op=mybir.AluOpType.mult)
                    nc.tensor.matmul(u_ps, sU1a, ht, start=True, stop=False)
                    nc.tensor.matmul(u_ps, sU1b, agg, start=False, stop=True)
                s2 = work.tile([HID, T], DT, tag="s2", bufs=3)
                nc.scalar.activation(s2, u_ps, Silu, bias=sbu, scale=1.0)

                # x = h + silu@U2 + b2u  (all accumulated in PSUM)
                x_ps = psA.tile([HID, T], f32, tag="xps", bufs=2)
                nc.tensor.matmul(x_ps, sU2, s2, start=True, stop=False)
                nc.tensor.matmul(x_ps, sb2u, sones_row, start=False, stop=False)
                nc.tensor.matmul(x_ps, sIdent, ht, start=False, stop=True)
                x_sb = x_full[:, base:base + T]
                nc.vector.tensor_copy(x_sb, x_ps)
                x2 = work.tile([HID, T], DT, tag="x2", bufs=2)
                nc.vector.tensor_tensor(out=x2, in0=x_sb, in1=x_sb,
                                        op=mybir.AluOpType.mult)
                # LN stats rows: band-select lhsT packs E[x] into psum row t
                # and E[x^2] into row NCH+t of one accumulating [64,T] bank
                hot = 2 * NCH - 1
                nc.tensor.matmul(st_ps[:, :], ssel[:, hot - t:hot - t + 2 * NCH],
                                 x_sb, start=(t == 0), stop=False)
                nc.tensor.matmul(st_ps[:, :],
                                 ssel[:, hot - NCH - t:hot - t + NCH],
                                 x2, start=False, stop=(t == NCH - 1))

            # ---------------- pass 1 ----------------
            load_chunk(0)
            for c in range(NCH + 1):
                if c < NCH:
                    if c + 1 < NCH:
                        load_chunk(c + 1)
                    phase_a(c)
                if c >= 1:
                    phase_b(c - 1)

            # ---------------- LN stats math ----------------
            # mr16 packs [mu | rstd] rows side by side -> one DRAM round trip
            # and one broadcast DMA per chunk in pass 2.
            mr16 = work.tile([NCH, 2 * T], DT, tag="mr16")
            nc.vector.tensor_copy(mr16[:, 0:T], st_ps[0:NCH, :])
            musq = work.tile([NCH, T], f32, tag="musq")
            nc.vector.tensor_tensor(out=musq, in0=mr16[:, 0:T],
                                    in1=mr16[:, 0:T],
                                    op=mybir.AluOpType.mult)
            var = work.tile([NCH, T], f32, tag="var")
            nc.vector.tensor_tensor(out=var, in0=st_ps[NCH:2 * NCH, :], in1=musq,
                                    op=mybir.AluOpType.subtract)
            sd32 = work.tile([NCH, T], f32, tag="sd32")
            nc.scalar.activation(sd32, var, Sqrt, bias=seps, scale=1.0)
            r32 = work.tile([NCH, T], f32, tag="r32")
            with nc.allow_low_precision(reason="rstd rows feed fp16 normalize"):
                nc.vector.reciprocal_approx_fast(out=r32, in_=sd32)
            nc.vector.tensor_copy(mr16[:, T:2 * T], r32)
            mr_dr = dramp.tile([NCH, 2 * T], DT)
            nc.sync.dma_start(out=mr_dr, in_=mr16)

            # ---------------- pass 2: normalize (no matmuls) ----------------
            # paired chunks: one [1..2,1024] broadcast DMA and 1024-wide DVE
            # ops per pair; per-partition layout mu(2q)|r(2q)|mu(2q+1)|r(2q+1)
            Ident = mybir.ActivationFunctionType.Identity
            for q in range(NCH // 2):
                base = q * 2 * T
                mr_bc = opool.tile([HID, 4 * T], DT, tag="mr_bc", bufs=4)
                src = mr_dr[2 * q:2 * q + 2, :]
                nc.gpsimd.dma_start(
                    out=mr_bc,
                    in_=bass.AP(tensor=src.tensor, offset=src.offset,
                                ap=[[0, HID]] + list(src.ap)))

                def pair_ap(col0):
                    s = mr_bc[:, col0:col0 + T]
                    return bass.AP(tensor=s.tensor, offset=s.offset,
                                   ap=[s.ap[0], [2 * T, 2], [1, T]])
                x_pair = x_full[:, base:base + 2 * T]
                t0 = work.tile([HID, 2 * T], DT, tag="t0", bufs=3)
                t0v = t0.rearrange("p (s t) -> p s t", t=T)
                nc.vector.tensor_tensor(out=t0v, in0=x_pair.rearrange(
                    "p (s t) -> p s t", t=T), in1=pair_ap(0),
                    op=mybir.AluOpType.subtract)
                t1 = work.tile([HID, 2 * T], DT, tag="t1", bufs=3)
                t1v = t1.rearrange("p (s t) -> p s t", t=T)
                nc.vector.tensor_tensor(out=t1v, in0=t0v, in1=pair_ap(T),
                                        op=mybir.AluOpType.mult)
                o = opool.tile([HID, 2 * T], DT, tag="o", bufs=5)
                # alternate the final per-channel affine between ACT and DVE
                if q % 2 == 0:
                    nc.scalar.activation(o, t1, Ident, bias=sb, scale=sg)
                else:
                    nc.vector.tensor_scalar(out=o, in0=t1, scalar1=sg,
                                            scalar2=sb,
                                            op0=mybir.AluOpType.mult,
                                            op1=mybir.AluOpType.add)
                nc.sync.dma_start(out=outT[:, base:base + 2 * T], in_=o)

    nc.compile()
    return nc


def _get_compiled(dt_name):
    global _compiled
    if _compiled is None:
        from concourse import mybir
        dt = {"bf16": mybir.dt.bfloat16, "fp16": mybir.dt.float16, "fp32": mybir.dt.float32}[dt_name]
        _compiled = _build_bass(dt)
    return _compiled


DT_NAME = "fp16"


def _sel_band(act_np):
    hot = 2 * NCH - 1
    sel = np.zeros((HID, 2 * 2 * NCH - 1), dtype=np.float32)
    sel[:, hot] = 1.0 / HID
    return sel.astype(act_np)


def kernel(**inputs):
    from concourse.bass_utils import run_bass_kernel_spmd

    h = np.asarray(inputs["h"], dtype=np.float32)
    coord = np.asarray(inputs["coord"], dtype=np.float32)
    msg_w1 = np.asarray(inputs["msg_w1"], dtype=np.float32)
    msg_b1 = np.asarray(inputs["msg_b1"], dtype=np.float32)
    msg_w2 = np.asarray(inputs["msg_w2"], dtype=np.float32)
    msg_b2 = np.asarray(inputs["msg_b2"], dtype=np.float32)
    upd_w1 = np.asarray(inputs["upd_w1"], dtype=np.float32)
    upd_b1 = np.asarray(inputs["upd_b1"], dtype=np.float32)
    upd_w2 = np.asarray(inputs["upd_w2"], dtype=np.float32)
    upd_b2 = np.asarray(inputs["upd_b2"], dtype=np.float32)
    ln_g = np.asarray(inputs["ln_g"], dtype=np.float32)
    ln_b = np.asarray(inputs["ln_b"], dtype=np.float32)

    import ml_dtypes
    act_np = {"bf16": ml_dtypes.bfloat16, "fp16": np.float16, "fp32": np.float32}[DT_NAME]

    W1a = msg_w1[:HID]
    W1b = msg_w1[HID:2 * HID]
    w1c = msg_w1[2 * HID]
    bias_u = upd_b1 + msg_b2 @ upd_w1[HID:2 * HID]
    W2s = msg_w2 / (2.0 * K)

    idx = np.arange(N)
    count = (np.minimum(idx, K) + np.minimum(N - 1 - idx, K)).astype(np.float32)
    fix = (2.0 * K) / count
    fixb = np.concatenate([fix[:K], fix[N - K:]]).reshape(1, 2 * K)

    wall = np.concatenate(
        [W1b, W1a, W2s @ upd_w1[HID:], upd_w1[:HID], upd_w2,
         np.eye(HID, dtype=np.float32), W2s, upd_w1[HID:]], axis=1)
    rows = np.concatenate(
        [w1c, -w1c, upd_b2]).reshape(1, 3 * HID)
    cols = np.stack([msg_b1, bias_u, ln_g, ln_b], axis=1)

    const = {
        "wall": np.ascontiguousarray(wall, dtype=act_np),
        "rows": np.ascontiguousarray(rows, dtype=act_np),
        "cols": np.ascontiguousarray(cols, dtype=np.float32),
        "fixb": np.ascontiguousarray(fixb, dtype=np.float32),
        "selb": _sel_band(act_np),
    }

    in_maps = []
    for b in range(B):
        m = dict(const)
        m["hT"] = np.ascontiguousarray(h[b].T, dtype=act_np)
        m["coordR"] = np.ascontiguousarray(coord[b].reshape(1, N), dtype=act_np)
        in_maps.append(m)

    nc = _get_compiled(DT_NAME)
    res = run_bass_kernel_spmd(nc, in_maps, core_ids=list(range(B)))
    global LAST_RESULTS
    LAST_RESULTS = res
    out = np.stack([np.asarray(res.results[b]["outT"], dtype=np.float32).T
                    for b in range(B)])
    return np.ascontiguousarray(out)


# revision 34
# speedup vs baseline: 1.0121x; 1.0121x over previous
"""Trainium2 Bass kernel for the LocalGNOBlock (windowed GNN message passing).

Math restructuring (vs the naive 12x full MLP evaluations):
  msg first layer is linear over concat([h_i, h_j, dc]):
      z_d[i] = (A - C)[i] + (B + C)[i+d] + b1,  d in {+-1..+-6}
  where A = h @ W1a, B = h @ W1b, C = coord x w1c (rank-1).
  The second msg layer is summed over edges BEFORE the matmul:
      agg_pre = (sum_d silu(z_d)) @ W2
  Aggregate divide-by-count folds into W2 (interior count == 12) with a
  6-column fixup at each sequence end.  LayerNorm stats are computed with
  ones-vector matmuls (channel dim lives on partitions).

Pass 2 (normalize) is matmul-free: mu/rstd rows are round-tripped through
DRAM and broadcast to 128 partitions with stride-0 DMA; the normalize is
two fp16 2x-mode tensor_tensor ops plus one 4x-mode tensor_scalar with
per-partition (g, b) operands.  Output is written fp16 and cast on host.

Sharding: batch dim B=8 -> one batch element per NeuronCore (no halo needed).
"""

import numpy as np

K = 6
HID = 128
N = 16384
B = 8
EPS = 1e-5
T = 512                 # token chunk (matmul + elementwise granularity)
NCH = N // T            # 32 chunks
OFF0 = 8                # D_full column of token 0 (even, for fp16 alignment)
NCOL = N + 2 * OFF0     # D_full width

# offsets: even offsets read D_A, odd offsets read D_B (shifted copy)
NEG_EVEN = [-6, -4, -2]
ODD = [-5, -3, -1, 1, 3, 5]
POS_EVEN = [2, 4, 6]
SEG_ORDER = NEG_EVEN + ODD + POS_EVEN  # 12 segments in Z

_compiled = None


def _build_bass(dt_act):
    import concourse.bacc as bacc
    import concourse.bass as bass
    import concourse.tile as tile
    from concourse import mybir

    f32 = mybir.dt.float32
    DT = dt_act

    nc = bacc.Bacc("TRN2", target_bir_lowering=False, debug=False)

    # ---- DRAM I/O ----
    hT = nc.dram_tensor("hT", [HID, N], DT, kind="ExternalInput")
    coordR = nc.dram_tensor("coordR", [1, N], DT, kind="ExternalInput")
    # wall packs the eight [128,128] weights: W1b W1a W2U U1a U2 ident W2s U1b
    wall = nc.dram_tensor("wall", [HID, 8 * HID], DT, kind="ExternalInput")
    # rows packs [w1c | -w1c | b2u] ; cols packs [b1 | bu | g | b] (f32)
    rows = nc.dram_tensor("rows", [1, 3 * HID], DT, kind="ExternalInput")
    cols = nc.dram_tensor("cols", [HID, 4], f32, kind="ExternalInput")
    fixb = nc.dram_tensor("fixb", [1, 2 * K], f32, kind="ExternalInput")  # 12/count head|tail
    # band-select matrix: hot column = 1/128 (stats row packing)
    selb = nc.dram_tensor("selb", [HID, 2 * 2 * NCH - 1], DT, kind="ExternalInput")
    outT = nc.dram_tensor("outT", [HID, N], DT, kind="ExternalOutput")

    Silu = mybir.ActivationFunctionType.Silu
    Sqrt = mybir.ActivationFunctionType.Sqrt

    with tile.TileContext(nc) as tc:
        with (
            tc.tile_pool(name="singles", bufs=1) as singles,
            tc.tile_pool(name="big", bufs=1) as big,
            tc.tile_pool(name="work", bufs=1) as work,
            tc.tile_pool(name="zpool", bufs=3) as zpool,
            tc.tile_pool(name="opool", bufs=3) as opool,
            tc.tile_pool(name="dramp", bufs=1, space="DRAM") as dramp,
            tc.tile_pool(name="psA", bufs=1, space="PSUM") as psA,
            tc.tile_pool(name="psB", bufs=1, space="PSUM") as psB,
            tc.tile_pool(name="psS", bufs=1, space="PSUM") as psS,
        ):
            # ---- constants into SBUF (few large DMAs, spread across queues) ----
            swall = singles.tile([HID, 8 * HID], DT)
            nc.scalar.dma_start(out=swall, in_=wall[:, :])
            sW1b = swall[:, 0 * HID:1 * HID]
            sW1a = swall[:, 1 * HID:2 * HID]
            sW2U = swall[:, 2 * HID:3 * HID]
            sU1a = swall[:, 3 * HID:4 * HID]
            sU2 = swall[:, 4 * HID:5 * HID]
            sIdent = swall[:, 5 * HID:6 * HID]
            sW2s = swall[:, 6 * HID:7 * HID]
            sU1b = swall[:, 7 * HID:8 * HID]
            srows = singles.tile([1, 3 * HID], DT)
            nc.gpsimd.dma_start(out=srows, in_=rows[:, :])
            sw1c = srows[:, 0:HID]
            sw1cn = srows[:, HID:2 * HID]
            sb2u = srows[:, 2 * HID:3 * HID]
            scols = singles.tile([HID, 4], f32)
            nc.gpsimd.dma_start(out=scols, in_=cols[:, :])
            sb1 = scols[:, 0:1]
            sbu = scols[:, 1:2]
            sg = scols[:, 2:3]
            sb = scols[:, 3:4]
            ssel = singles.tile([HID, 2 * 2 * NCH - 1], DT)
            nc.scalar.dma_start(out=ssel, in_=selb[:, :])

            def bcast_rows(a, p=HID):
                return bass.AP(tensor=a.tensor, offset=a.offset,
                               ap=[[0, p]] + list(a.ap[1:]))

            # broadcast [1,12] -> [128,12] fix tile (head | tail)
            sfix = singles.tile([HID, 2 * K], f32)
            nc.gpsimd.dma_start(out=sfix, in_=bcast_rows(fixb[0:1, :]))
            sfixf = sfix[:, 0:K]
            sfixl = sfix[:, K:2 * K]
            sones_row = singles.tile([1, T], DT)
            nc.vector.memset(sones_row, 1.0)
            seps = singles.tile([NCH, 1], f32)
            nc.vector.memset(seps, float(EPS))

            # ---- big persistent buffers ----
            D_A = big.tile([HID, NCOL], DT)      # token j at col OFF0 + j
            D_B = big.tile([HID, NCOL], DT)      # token j at col OFF0 + 1 + j
            x_full = big.tile([HID, N], DT)
            # zero halo columns of D so boundary silu stays finite
            nc.vector.memset(D_A[:, 0:OFF0], 0.0)
            nc.vector.memset(D_A[:, OFF0 + N:NCOL], 0.0)
            nc.vector.memset(D_B[:, 0:OFF0 + 1], 0.0)
            nc.vector.memset(D_B[:, OFF0 + 1 + N:NCOL], 0.0)

            # LN stats: rows [0:32] = E[x]/chunk, [32:64] = E[x^2]/chunk
            st_ps = psS.tile([2 * NCH, T], f32)

            hts = {}
            crd = {}

            def load_chunk(c):
                ht = work.tile([HID, T], DT, tag="ht", bufs=6)
                nc.sync.dma_start(out=ht, in_=hT[:, c * T:(c + 1) * T])
                co = work.tile([1, T], DT, tag="co", bufs=6)
                nc.sync.dma_start(out=co, in_=coordR[:, c * T:(c + 1) * T])
                hts[c] = ht
                crd[c] = co

            def phase_a(c):
                # D chunk = W1b.T @ h  +  w1c x coord   (PSUM accumulate)
                d_ps = psA.tile([HID, T], f32, tag="de", bufs=2)
                nc.tensor.matmul(d_ps, sW1b, hts[c], start=True, stop=False)
                nc.tensor.matmul(d_ps, sw1c, crd[c], start=False, stop=True)
                col = OFF0 + c * T
                nc.vector.tensor_copy(D_A[:, col:col + T], d_ps)
                # shifted copy via SBUF->SBUF DMA: gpsimd COPY would contend
                # with the DVE for the shared POOL SBUF port and halve DVE
                # throughput on overlapping ops
                nc.sync.dma_start(
                    out=D_B[:, col + 1:col + 1 + T], in_=D_A[:, col:col + T])

            def seg_in1(tile_ap, col, nseg):
                # [128, nseg, T] AP over D with outer column-stride 2
                s = tile_ap[:, col:col + T]
                return bass.AP(tensor=s.tensor, offset=s.offset,
                               ap=[s.ap[0], [2, nseg], [1, T]])

            def phase_b(t):
                ht, co = hts[t], crd[t]
                # E chunk = W1a.T @ h - w1c x coord
                e_ps = psA.tile([HID, T], f32, tag="de", bufs=2)
                nc.tensor.matmul(e_ps, sW1a, ht, start=True, stop=False)
                nc.tensor.matmul(e_ps, sw1cn, co, start=False, stop=True)
                e_sb = work.tile([HID, T], DT, tag="esb", bufs=3)
                nc.vector.tensor_copy(e_sb, e_ps)

                # Z: 12 segments of E + shifted D, 3 stride-2 groups
                z = zpool.tile([HID, 12 * T], DT, tag="z")
                zv = z.rearrange("p (s t) -> p s t", t=T)

                def e_bc(nseg):
                    return bass.AP(tensor=e_sb.tensor, offset=e_sb.offset,
                                   ap=[e_sb.ap[0], [0, nseg], [1, T]])
                base = t * T
                groups = [
                    (D_A, OFF0 + base + NEG_EVEN[0], 0, 3),
                    (D_B, OFF0 + 1 + base + ODD[0], 3, 6),
                    (D_A, OFF0 + base + POS_EVEN[0], 9, 3),
                ]
                for dbuf, col, s0, nseg in groups:
                    nc.vector.tensor_tensor(
                        out=zv[:, s0:s0 + nseg, :],
                        in0=e_bc(nseg), in1=seg_in1(dbuf, col, nseg),
                        op=mybir.AluOpType.add)

                # silu over all 12 segments at once (bias = msg_b1)
                nc.scalar.activation(z, z, Silu, bias=sb1, scale=1.0)

                # zero invalid boundary columns (torn edges of the sequence)
                if t == 0:
                    for s, d in enumerate(SEG_ORDER):
                        if d < 0:
                            nc.vector.memset(zv[:, s, 0:-d], 0.0)
                if t == NCH - 1:
                    for s, d in enumerate(SEG_ORDER):
                        if d > 0:
                            nc.vector.memset(zv[:, s, T - d:T], 0.0)

                # update-MLP first layer.  For interior chunks the agg path is
                # algebraically folded: U1b.T @ (W2s.T @ sum_s silu(z_s)) ==
                # (W2s @ U1b).T @ sum_s silu(z_s), so the 12 segment matmuls
                # accumulate straight into u_ps with lhsT = W2U.  Boundary
                # chunks need the 12/count fixup on agg, so they keep the
                # explicit agg tensor.
                u_ps = psA.tile([HID, T], f32, tag="upd", bufs=2)
                if 0 < t < NCH - 1:
                    nc.tensor.matmul(u_ps, sU1a, ht, start=True, stop=False)
                    for s in range(12):
                        nc.tensor.matmul(u_ps, sW2U, zv[:, s, :],
                                         start=False, stop=(s == 11))
                else:
                    a_ps = psB.tile([HID, T], f32, tag="agg", bufs=1)
                    for s in range(12):
                        nc.tensor.matmul(a_ps, sW2s, zv[:, s, :],
                                         start=(s == 0), stop=(s == 11))
                    agg = work.tile([HID, T], DT, tag="agg_sb", bufs=2)
                    nc.vector.tensor_copy(agg, a_ps)
                    if t == 0:
                        nc.vector.tensor_tensor(out=agg[:, 0:K], in0=a_ps[:, 0:K],
                                                in1=sfixf, op=mybir.AluOpType.mult)
                    if t == NCH - 1:
                        nc.vector.tensor_tensor(out=agg[:, T - K:T],
                                                in0=a_ps[:, T - K:T],
                                                in1=sfixl, op=mybir.AluOpType.mult)
                    nc.tensor.matmul(u_ps, sU1a, ht, start=True, stop=False)
                    nc.tensor.matmul(u_ps, sU1b, agg, start=False, stop=True)
                s2 = work.tile([HID, T], DT, tag="s2", bufs=3)
                nc.scalar.activation(s2, u_ps, Silu, bias=sbu, scale=1.0)

                # x = h + silu@U2 + b2u  (all accumulated in PSUM)
                x_ps = psA.tile([HID, T], f32, tag="xps", bufs=2)
                nc.tensor.matmul(x_ps, sU2, s2, start=True, stop=False)
                nc.tensor.matmul(x_ps, sb2u, sones_row, start=False, stop=False)
                nc.tensor.matmul(x_ps, sIdent, ht, start=False, stop=True)
                x_sb = x_full[:, base:base + T]
                nc.vector.tensor_copy(x_sb, x_ps)
                x2 = work.tile([HID, T], DT, tag="x2", bufs=2)
                nc.vector.tensor_tensor(out=x2, in0=x_sb, in1=x_sb,
                                        op=mybir.AluOpType.mult)
                # LN stats rows: band-select lhsT packs E[x] into psum row t
                # and E[x^2] into row NCH+t of one accumulating [64,T] bank
                hot = 2 * NCH - 1
                nc.tensor.matmul(st_ps[:, :], ssel[:, hot - t:hot - t + 2 * NCH],
                                 x_sb, start=(t == 0), stop=False)
                nc.tensor.matmul(st_ps[:, :],
                                 ssel[:, hot - NCH - t:hot - t + NCH],
                                 x2, start=False, stop=(t == NCH - 1))

            # ---------------- pass 1 ----------------
            load_chunk(0)
            for c in range(NCH + 1):
                if c < NCH:
                    if c + 1 < NCH:
                        load_chunk(c + 1)
                    phase_a(c)
                if c >= 1:
                    phase_b(c - 1)

            # ---------------- LN stats math ----------------
            # mr16 packs [mu | rstd] rows side by side -> one DRAM round trip
            # and one broadcast DMA per chunk in pass 2.
            mr16 = work.tile([NCH, 2 * T], DT, tag="mr16")
            nc.vector.tensor_copy(mr16[:, 0:T], st_ps[0:NCH, :])
            musq = work.tile([NCH, T], f32, tag="musq")
            nc.vector.tensor_tensor(out=musq, in0=mr16[:, 0:T],
                                    in1=mr16[:, 0:T],
                                    op=mybir.AluOpType.mult)
            var = work.tile([NCH, T], f32, tag="var")
            nc.vector.tensor_tensor(out=var, in0=st_ps[NCH:2 * NCH, :], in1=musq,
                                    op=mybir.AluOpType.subtract)
            sd32 = work.tile([NCH, T], f32, tag="sd32")
            nc.scalar.activation(sd32, var, Sqrt, bias=seps, scale=1.0)
            r32 = work.tile([NCH, T], f32, tag="r32")
            with nc.allow_low_precision(reason="rstd rows feed fp16 normalize"):
                nc.vector.reciprocal_approx_fast(out=r32, in_=sd32)
            nc.vector.tensor_copy(mr16[:, T:2 * T], r32)
            mr_dr = dramp.tile([NCH, 2 * T], DT)
            nc.sync.dma_start(out=mr_dr, in_=mr16)

            # ---------------- pass 2: normalize (no matmuls) ----------------
            # paired chunks: one [1..2,1024] broadcast DMA and 1024-wide DVE
            # ops per pair; per-partition layout mu(2q)|r(2q)|mu(2q+1)|r(2q+1)
            Ident = mybir.ActivationFunctionType.Identity
            for q in range(NCH // 2):
                base = q * 2 * T
                mr_bc = opool.tile([HID, 4 * T], DT, tag="mr_bc", bufs=4)
                src = mr_dr[2 * q:2 * q + 2, :]
                nc.gpsimd.dma_start(
                    out=mr_bc,
                    in_=bass.AP(tensor=src.tensor, offset=src.offset,
                                ap=[[0, HID]] + list(src.ap)))

                def pair_ap(col0):
                    s = mr_bc[:, col0:col0 + T]
                    return bass.AP(tensor=s.tensor, offset=s.offset,
                                   ap=[s.ap[0], [2 * T, 2], [1, T]])
                x_pair = x_full[:, base:base + 2 * T]
                t0 = work.tile([HID, 2 * T], DT, tag="t0", bufs=3)
                t0v = t0.rearrange("p (s t) -> p s t", t=T)
                nc.vector.tensor_tensor(out=t0v, in0=x_pair.rearrange(
                    "p (s t) -> p s t", t=T), in1=pair_ap(0),
                    op=mybir.AluOpType.subtract)
                t1 = work.tile([HID, 2 * T], DT, tag="t1", bufs=3)
                t1v = t1.rearrange("p (s t) -> p s t", t=T)
                nc.vector.tensor_tensor(out=t1v, in0=t0v, in1=pair_ap(T),
                                        op=mybir.AluOpType.mult)
                o = opool.tile([HID, 2 * T], DT, tag="o", bufs=5)
                # alternate the final per-channel affine between ACT and DVE
                if q % 2 == 0:
                    nc.scalar.activation(o, t1, Ident, bias=sb, scale=sg)
                else:
                    nc.vector.tensor_scalar(out=o, in0=t1, scalar1=sg,
                                            scalar2=sb,
                                            op0=mybir.AluOpType.mult,
                                            op1=mybir.AluOpType.add)
                nc.sync.dma_start(out=outT[:, base:base + 2 * T], in_=o)

    nc.compile()
    return nc


def _get_compiled(dt_name):
    global _compiled
    if _compiled is None:
        from concourse import mybir
        dt = {"bf16": mybir.dt.bfloat16, "fp16": mybir.dt.float16, "fp32": mybir.dt.float32}[dt_name]
        _compiled = _build_bass(dt)
    return _compiled


DT_NAME = "fp16"


def _sel_band(act_np):
    hot = 2 * NCH - 1
    sel = np.zeros((HID, 2 * 2 * NCH - 1), dtype=np.float32)
    sel[:, hot] = 1.0 / HID
    return sel.astype(act_np)


def kernel(**inputs):
    from concourse.bass_utils import run_bass_kernel_spmd

    h = np.asarray(inputs["h"], dtype=np.float32)
    coord = np.asarray(inputs["coord"], dtype=np.float32)
    msg_w1 = np.asarray(inputs["msg_w1"], dtype=np.float32)
    msg_b1 = np.asarray(inputs["msg_b1"], dtype=np.float32)
    msg_w2 = np.asarray(inputs["msg_w2"], dtype=np.float32)
    msg_b2 = np.asarray(inputs["msg_b2"], dtype=np.float32)
    upd_w1 = np.asarray(inputs["upd_w1"], dtype=np.float32)
    upd_b1 = np.asarray(inputs["upd_b1"], dtype=np.float32)
    upd_w2 = np.asarray(inputs["upd_w2"], dtype=np.float32)
    upd_b2 = np.asarray(inputs["upd_b2"], dtype=np.float32)
    ln_g = np.asarray(inputs["ln_g"], dtype=np.float32)
    ln_b = np.asarray(inputs["ln_b"], dtype=np.float32)

    import ml_dtypes
    act_np = {"bf16": ml_dtypes.bfloat16, "fp16": np.float16, "fp32": np.float32}[DT_NAME]

    W1a = msg_w1[:HID]
    W1b = msg_w1[HID:2 * HID]
    w1c = msg_w1[2 * HID]
    bias_u = upd_b1 + msg_b2 @ upd_w1[HID:2 * HID]
    W2s = msg_w2 / (2.0 * K)

    idx = np.arange(N)
    count = (np.minimum(idx, K) + np.minimum(N - 1 - idx, K)).astype(np.float32)
    fix = (2.0 * K) / count
    fixb = np.concatenate([fix[:K], fix[N - K:]]).reshape(1, 2 * K)

    wall = np.concatenate(
        [W1b, W1a, W2s @ upd_w1[HID:], upd_w1[:HID], upd_w2,
         np.eye(HID, dtype=np.float32), W2s, upd_w1[HID:]], axis=1)
    rows = np.concatenate(
        [w1c, -w1c, upd_b2]).reshape(1, 3 * HID)
    cols = np.stack([msg_b1, bias_u, ln_g, ln_b], axis=1)

    const = {
        "wall": np.ascontiguousarray(wall, dtype=act_np),
        "rows": np.ascontiguousarray(rows, dtype=act_np),
        "cols": np.ascontiguousarray(cols, dtype=np.float32),
        "fixb": np.ascontiguousarray(fixb, dtype=np.float32),
        "selb": _sel_band(act_np),
    }

    in_maps = []
    for b in range(B):
        m = dict(const)
        m["hT"] = np.ascontiguousarray(h[b].T, dtype=act_np)
        m["coordR"] = np.ascontiguousarray(coord[b].reshape(1, N), dtype=act_np)
        in_maps.append(m)

    nc = _get_compiled(DT_NAME)
    res = run_bass_kernel_spmd(nc, in_maps, core_ids=list(range(B)))
    global LAST_RESULTS
    LAST_RESULTS = res
    out = np.stack([np.asarray(res.results[b]["outT"], dtype=np.float32).T
                    for b in range(B)])
    return np.ascontiguousarray(out)
